# revision 1
# baseline (speedup 1.0000x reference)
"""Trainium2 Bass kernel: batched chamfer-style metric (nn_Metric_56985626083917).

Reference computation per batch b (B=8, N=M=4096, D=3):
    sqd[n,m] = |pred_n - gt_m|^2   (clamped >= 0)
    dist1 = sqrt(min_m sqd)  [N] ; dist2 = sqrt(min_n sqd)  [M]
    loss_b = mean(dist1)+mean(dist2) + 3*(mean(top2048(dist1))+mean(top2048(dist2)))
    out = mean_b loss_b

Strategy: data-parallel, one batch per NeuronCore (8 cores).
Per core the device computes zt[n,m] = -sqd[n,m] via a single K=16 fp16
matmul with error-compensated hi/lo splits (fp32-grade accuracy at full
fp16 PE rate):
    zt = sum_c 2*p_c*g_c - |p|^2 - |g|^2
slots: per coord c 4 products (ph*gh, ph*gl, pl*gh, pl*gl), plus 2 slots for
-|p|^2 (hi/lo vs ones) and 2 slots for -|g|^2.

slots: per coord c 3 products (ph*gh, ph*gl, pl*gh; pl*gl ~ 2^-22 dropped),
plus 2 slots for -|p|^2 (hi/lo vs ones) and 2 slots for -|g|^2 -> K=13.

Main loop per 128-row pred tile (32 iterations):
    PE   : 8 matmuls -> PSUM [128, 4096] fp32 (zt tile)
    DVE  : tensor_tensor max (PSUM, SBUF) -> running column-max (dist2 path)
           tensor_reduce max (PSUM)       -> per-row max        (dist1 path)
Tail: two partition folds (128 -> 32) with DMA realignment (the BIR
verifier requires equal base partitions for two-SBUF-input TensorTensor).
Device outputs row maxes [128, 32] and partially-folded column maxes
[32, 4096]; the host (O(N) work) finishes the fold, does relu/sqrt,
means, and exact top-k via np.partition, then averages the 8 losses.
"""

import os
import sys

import numpy as np

for _p in ("/opt/trn_rl_repo",):
    if os.path.isdir(_p) and _p not in sys.path:
        sys.path.insert(0, _p)

import concourse.bass as bass  # noqa: E402
import concourse.mybir as mybir  # noqa: E402
import concourse.tile as tile  # noqa: E402
from concourse import bacc  # noqa: E402
from concourse.bass_utils import run_bass_kernel_spmd  # noqa: E402

B = 8
N = 4096  # pred points per batch
M = 4096  # gt points per batch
P = 128  # partitions
KSLOTS = 13
NTILE = N // P  # 32
MCHUNK = 4096  # gt columns processed per pred tile iteration
NCHUNK = M // MCHUNK
MM_N = 512  # moving free dim per matmul (<= 1 PSUM bank)
K1 = N // 2  # top-k count (PERCENT=0.5)
WEIGHT = 3.0

F16 = mybir.dt.float16
F32 = mybir.dt.float32
Alu = mybir.AluOpType

LAST_RESULT = None
_CACHE = {}


def _build_nc(reps=1):
    nc = bacc.Bacc(
        "TRN2", target_bir_lowering=False, debug=False, num_devices=B
    )
    a_in = nc.dram_tensor("A", [KSLOTS, N], F16, kind="ExternalInput")
    g_in = nc.dram_tensor("G", [KSLOTS, M], F16, kind="ExternalInput")
    r1_out = nc.dram_tensor("R1OUT", [P, NTILE * NCHUNK], F32, kind="ExternalOutput")
    c2_out = nc.dram_tensor("C2OUT", [32, M], F32, kind="ExternalOutput")

    with tile.TileContext(nc) as tc:
        for _ in range(reps):
            _body(nc, tc, a_in, g_in, r1_out, c2_out)
    nc.compile()
    return nc


def _body(nc, tc, a_in, g_in, r1_out, c2_out):
    from contextlib import ExitStack

    with ExitStack() as ctx:
        io = ctx.enter_context(tc.tile_pool(name="io", bufs=1))
        runp = ctx.enter_context(tc.tile_pool(name="run", bufs=1))
        smallp = ctx.enter_context(tc.tile_pool(name="small", bufs=1))

        A = io.tile([KSLOTS, N], F16)
        G = io.tile([KSLOTS, M], F16)
        nc.sync.dma_start(out=A, in_=a_in[:])
        nc.sync.dma_start(out=G, in_=g_in[:])

        run2 = runp.tile([P, M], F32)
        nc.vector.memset(run2, -3.0e38)
        Rpart = smallp.tile([P, NTILE * NCHUNK], F32, name="Rpart")

        # ---------- main loop ----------
        ps_bufs = 2 if MCHUNK <= 2048 else 1
        with tc.tile_pool(name="ps_main", bufs=ps_bufs, space="PSUM") as psum:
            for i in range(NTILE):
                for jj in range(NCHUNK):
                    ps = psum.tile([P, MCHUNK], F32)
                    for kk in range(MCHUNK // MM_N):
                        nc.tensor.matmul(
                            ps[:, kk * MM_N : (kk + 1) * MM_N],
                            A[:, i * P : (i + 1) * P],
                            G[:, jj * MCHUNK + kk * MM_N : jj * MCHUNK + (kk + 1) * MM_N],
                            start=True,
                            stop=True,
                        )
                    # colmax accumulate (dist2 path) — read PSUM directly
                    nc.vector.tensor_tensor(
                        run2[:, jj * MCHUNK : (jj + 1) * MCHUNK],
                        ps,
                        run2[:, jj * MCHUNK : (jj + 1) * MCHUNK],
                        op=Alu.max,
                    )
                    # rowmax partial (dist1 path)
                    s = i * NCHUNK + jj
                    nc.vector.tensor_reduce(
                        out=Rpart[:, s : s + 1],
                        in_=ps,
                        axis=mybir.AxisListType.X,
                        op=Alu.max,
                    )

        # ---------- tail: fold run2 across partitions (128 -> 32) ----------
        # TensorTensor requires equal base partitions for both SBUF inputs,
        # so DMA-realign the upper half to partition 0 before each fold.
        # Host finishes the 32 -> 1 fold.
        tmp = runp.tile([64, M], F32, name="ftmp")
        nc.sync.dma_start(out=tmp, in_=run2[64:128, :])
        nc.vector.tensor_tensor(run2[0:64, :], run2[0:64, :], tmp, op=Alu.max)
        nc.sync.dma_start(out=tmp[0:32, :], in_=run2[32:64, :])
        nc.vector.tensor_tensor(
            run2[0:32, :], run2[0:32, :], tmp[0:32, :], op=Alu.max
        )

        nc.sync.dma_start(out=r1_out[:], in_=Rpart)
        nc.sync.dma_start(out=c2_out[:], in_=run2[0:32, :])


def _split16(x):
    hi = x.astype(np.float16)
    lo = (x - hi.astype(np.float64)).astype(np.float16)
    return hi, lo


def _prep(pred, gt):
    """Build the [16, 4096] fp16 stationary/moving operand matrices."""
    p = pred.astype(np.float64)
    g = gt.astype(np.float64)
    ph, pl = _split16(p)  # [N,3] each
    gh, gl = _split16(g)
    pt = ph.astype(np.float64) + pl.astype(np.float64)
    gt_ = gh.astype(np.float64) + gl.astype(np.float64)
    pn = (pt * pt).sum(-1)  # [N]
    gn = (gt_ * gt_).sum(-1)  # [M]
    pnh, pnl = _split16(-pn)
    gnh, gnl = _split16(-gn)

    A = np.zeros((KSLOTS, N), np.float16)
    G = np.zeros((KSLOTS, M), np.float16)
    for c in range(3):
        r = 3 * c
        # (ph+pl)*(gh+gl) ~= ph*gh + ph*gl + pl*gh  (pl*gl ~ 2^-22, dropped)
        A[r + 0] = 2.0 * ph[:, c]
        A[r + 1] = 2.0 * ph[:, c]
        A[r + 2] = 2.0 * pl[:, c]
        G[r + 0] = gh[:, c]
        G[r + 1] = gl[:, c]
        G[r + 2] = gh[:, c]
    A[9] = pnh
    A[10] = pnl
    G[9] = 1.0
    G[10] = 1.0
    A[11] = 1.0
    A[12] = 1.0
    G[11] = gnh
    G[12] = gnl
    return A, G


def _get_nc():
    if "nc" not in _CACHE:
        _CACHE["nc"] = _build_nc()
    return _CACHE["nc"]


def kernel(pred_pc, gt_pc):
    global LAST_RESULT
    pred_pc = np.asarray(pred_pc)
    gt_pc = np.asarray(gt_pc)
    nc = _get_nc()
    in_maps = []
    for b in range(B):
        A, G = _prep(pred_pc[b], gt_pc[b])
        in_maps.append({"A": A, "G": G})
    res = run_bass_kernel_spmd(nc, in_maps, list(range(B)))
    LAST_RESULT = res
    losses = []
    for b in range(B):
        r1 = np.asarray(res.results[b]["R1OUT"], np.float32)
        c2 = np.asarray(res.results[b]["C2OUT"], np.float32).max(axis=0)  # [4096]
        # rowmax: combine NCHUNK partials per point
        r1 = r1.reshape(P, NTILE, NCHUNK).max(axis=2)  # [P, NTILE]
        d1 = np.sqrt(np.maximum(-r1, 0.0)).reshape(-1)  # [4096]
        d2 = np.sqrt(np.maximum(-c2, 0.0))  # [4096]
        loss = 0.0
        for d in (d1, d2):
            topk = np.partition(d, d.size - K1)[d.size - K1 :]
            loss += d.mean() + WEIGHT * topk.mean()
        losses.append(loss)
    return np.array(np.mean(losses), dtype=np.float32)



# revision 3
# speedup vs baseline: 4.3583x; 4.3583x over previous
"""Trainium2 Bass kernel: batched chamfer-style metric (nn_Metric_56985626083917).

Reference computation per batch b (B=8, N=M=4096, D=3):
    sqd[n,m] = |pred_n - gt_m|^2   (clamped >= 0)
    dist1 = sqrt(min_m sqd)  [N] ; dist2 = sqrt(min_n sqd)  [M]
    loss_b = mean(dist1)+mean(dist2) + 3*(mean(top2048(dist1))+mean(top2048(dist2)))
    out = mean_b loss_b

Strategy: data-parallel, one batch per NeuronCore (8 cores).
Per core the device computes zt[n,m] = -sqd[n,m] via K=13 fp16 matmuls with
error-compensated hi/lo splits (fp32-grade accuracy at full fp16 PE rate):
    zt = sum_c 2*p_c*g_c - |p|^2 - |g|^2
slots: per coord c 3 products (ph*gh, ph*gl, pl*gh; pl*gl ~ 2^-22 dropped),
plus 2 slots for -|p|^2 (hi/lo vs ones) and 2 slots for -|g|^2 -> K=13.

Main loop is a HARDWARE loop (tc.For_i) over 32 pred tiles of 128 rows;
per tile two 2048-wide PSUM halves are double-buffered:
    PE  : 4 matmuls -> PSUM [128, 2048] fp32 (zt half-tile)
    DVE : tensor_reduce max over free axis -> Rpart column (dist1 path)
    ACT : tensor_tensor max (PSUM, run2) -> run2 (dist2 path, via nc.any)
The hardware loop keeps the BIR tiny (~30 instructions vs ~330 unrolled),
which shrinks the NEFF and the per-run compile/dispatch cost ~10x.

Device outputs raw row maxes [128, 64] and unfolded column maxes
[128, 4096]; the host (O(N) work) folds partitions, does relu/sqrt, means,
and exact top-k via np.partition, then averages the 8 losses.
"""

import os
import sys

import numpy as np

for _p in ("/opt/trn_rl_repo",):
    if os.path.isdir(_p) and _p not in sys.path:
        sys.path.insert(0, _p)

import concourse.bass as bass  # noqa: E402
import concourse.mybir as mybir  # noqa: E402
import concourse.tile as tile  # noqa: E402
from concourse import bacc  # noqa: E402
from concourse.bass import ds, ts  # noqa: E402
from concourse.bass_utils import run_bass_kernel_spmd  # noqa: E402

B = 8
N = 4096  # pred points per batch
M = 4096  # gt points per batch
P = 128  # partitions
KSLOTS = 13
NTILE = N // P  # 32
MCHUNK = 2048  # gt columns per PSUM half
NCHUNK = M // MCHUNK  # 2
MM_N = 512  # moving free dim per matmul (<= 1 PSUM bank)
K1 = N // 2  # top-k count (PERCENT=0.5)
WEIGHT = 3.0

F16 = mybir.dt.float16
F32 = mybir.dt.float32
Alu = mybir.AluOpType
X = mybir.AxisListType.X

LAST_RESULT = None
_CACHE = {}


def _build_nc(reps=1):
    nc = bacc.Bacc(
        "TRN2", target_bir_lowering=False, debug=False, num_devices=B
    )
    a_in = nc.dram_tensor("A", [KSLOTS, N], F16, kind="ExternalInput")
    g_in = nc.dram_tensor("G", [KSLOTS, M], F16, kind="ExternalInput")
    r1_out = nc.dram_tensor("R1OUT", [P, NTILE * NCHUNK], F32, kind="ExternalOutput")
    c2_out = nc.dram_tensor("C2OUT", [P, M], F32, kind="ExternalOutput")

    with tile.TileContext(nc) as tc:
        for _ in range(reps):
            _body(nc, tc, a_in, g_in, r1_out, c2_out)
    nc.compile()
    return nc


def _body(nc, tc, a_in, g_in, r1_out, c2_out):
    from contextlib import ExitStack

    with ExitStack() as ctx:
        io = ctx.enter_context(tc.tile_pool(name="io", bufs=1))
        statp = ctx.enter_context(tc.tile_pool(name="stat", bufs=2))

        G = io.tile([KSLOTS, M], F16)
        nc.sync.dma_start(out=G, in_=g_in[:])

        run2 = io.tile([P, M], F32)
        nc.vector.memset(run2, -3.0e38)
        Rpart = io.tile([P, NTILE * NCHUNK], F32, name="Rpart")

        with tc.tile_pool(name="ps_main", bufs=2, space="PSUM") as psum:
            with tc.For_i(0, NTILE, 1) as i:
                # ldweights needs a static SBUF offset, so stage this pred
                # tile's stationary slab into a fixed (double-buffered) tile
                Astat = statp.tile([KSLOTS, P], F16)
                nc.sync.dma_start(out=Astat, in_=a_in[:, ts(i, P)])
                for j in range(NCHUNK):
                    ps = psum.tile([P, MCHUNK], F32)
                    for kk in range(MCHUNK // MM_N):
                        c0 = j * MCHUNK + kk * MM_N
                        nc.tensor.matmul(
                            ps[:, kk * MM_N : (kk + 1) * MM_N],
                            Astat,
                            G[:, c0 : c0 + MM_N],
                            start=True,
                            stop=True,
                        )
                    # rowmax partial (dist1 path) on DVE
                    nc.vector.tensor_reduce(
                        out=Rpart[:, ds(i * NCHUNK + j, 1)],
                        in_=ps,
                        axis=X,
                        op=Alu.max,
                    )
                    # colmax accumulate (dist2 path); nc.any routes to the
                    # idle ACT engine so it overlaps the DVE reduce
                    nc.any.tensor_tensor(
                        run2[:, j * MCHUNK : (j + 1) * MCHUNK],
                        ps,
                        run2[:, j * MCHUNK : (j + 1) * MCHUNK],
                        op=Alu.max,
                    )

        nc.sync.dma_start(out=r1_out[:], in_=Rpart)
        nc.sync.dma_start(out=c2_out[:], in_=run2)


def _split16(x):
    hi = x.astype(np.float16)
    lo = (x - hi.astype(np.float64)).astype(np.float16)
    return hi, lo


def _prep(pred, gt):
    """Build the [13, 4096] fp16 stationary/moving operand matrices."""
    p = pred.astype(np.float64)
    g = gt.astype(np.float64)
    ph, pl = _split16(p)  # [N,3] each
    gh, gl = _split16(g)
    pt = ph.astype(np.float64) + pl.astype(np.float64)
    gt_ = gh.astype(np.float64) + gl.astype(np.float64)
    pn = (pt * pt).sum(-1)  # [N]
    gn = (gt_ * gt_).sum(-1)  # [M]
    pnh, pnl = _split16(-pn)
    gnh, gnl = _split16(-gn)

    A = np.zeros((KSLOTS, N), np.float16)
    G = np.zeros((KSLOTS, M), np.float16)
    for c in range(3):
        r = 3 * c
        # (ph+pl)*(gh+gl) ~= ph*gh + ph*gl + pl*gh  (pl*gl ~ 2^-22, dropped)
        A[r + 0] = 2.0 * ph[:, c]
        A[r + 1] = 2.0 * ph[:, c]
        A[r + 2] = 2.0 * pl[:, c]
        G[r + 0] = gh[:, c]
        G[r + 1] = gl[:, c]
        G[r + 2] = gh[:, c]
    A[9] = pnh
    A[10] = pnl
    G[9] = 1.0
    G[10] = 1.0
    A[11] = 1.0
    A[12] = 1.0
    G[11] = gnh
    G[12] = gnl
    return A, G


def _get_nc():
    if "nc" not in _CACHE:
        _CACHE["nc"] = _build_nc()
    return _CACHE["nc"]


def kernel(pred_pc, gt_pc):
    global LAST_RESULT
    pred_pc = np.asarray(pred_pc)
    gt_pc = np.asarray(gt_pc)
    nc = _get_nc()
    in_maps = []
    for b in range(B):
        A, G = _prep(pred_pc[b], gt_pc[b])
        in_maps.append({"A": A, "G": G})
    res = run_bass_kernel_spmd(nc, in_maps, list(range(B)))
    LAST_RESULT = res
    losses = []
    for b in range(B):
        r1 = np.asarray(res.results[b]["R1OUT"], np.float32)
        c2 = np.asarray(res.results[b]["C2OUT"], np.float32).max(axis=0)  # [4096]
        # rowmax: combine NCHUNK partials per point
        r1 = r1.reshape(P, NTILE, NCHUNK).max(axis=2)  # [P, NTILE]
        d1 = np.sqrt(np.maximum(-r1, 0.0)).reshape(-1)  # [4096]
        d2 = np.sqrt(np.maximum(-c2, 0.0))  # [4096]
        loss = 0.0
        for d in (d1, d2):
            topk = np.partition(d, d.size - K1)[d.size - K1 :]
            loss += d.mean() + WEIGHT * topk.mean()
        losses.append(loss)
    return np.array(np.mean(losses), dtype=np.float32)


# revision 10
# speedup vs baseline: 6.6110x; 1.5169x over previous
"""Trainium2 Bass kernel: batched chamfer-style metric (nn_Metric_56985626083917).

Reference computation per batch b (B=8, N=M=4096, D=3):
    sqd[n,m] = |pred_n - gt_m|^2   (clamped >= 0)
    dist1 = sqrt(min_m sqd)  [N] ; dist2 = sqrt(min_n sqd)  [M]
    loss_b = mean(dist1)+mean(dist2) + 3*(mean(top2048(dist1))+mean(top2048(dist2)))
    out = mean_b loss_b

Strategy: data-parallel, one batch per NeuronCore (8 cores).
Per core the device computes zt[n,m] = -sqd[n,m] via K=13 fp16 matmuls with
error-compensated hi/lo splits (fp32-grade accuracy at full fp16 PE rate):
    zt = sum_c 2*p_c*g_c - |p|^2 - |g|^2
slots: per coord c 3 products (ph*gh, ph*gl, pl*gh; pl*gl ~ 2^-22 dropped),
plus 2 slots for -|p|^2 (hi/lo vs ones) and 2 slots for -|g|^2 -> K=13.

Main loop: statically unrolled over 32 pred tiles x 2 PSUM halves of 2048
gt columns (double-buffered), wrapped in a HARDWARE loop (tc.For_i) over
`reps` so benchmark repetitions re-execute the same instructions instead
of growing the NEFF:
    PE  : 4 matmuls -> PSUM [128, 2048] fp32 (zt half-tile)
    ACT : copy PSUM -> SBUF fp16 (feeds the dist2 path)
    DVE : tensor_reduce max over PSUM -> rowmax column (dist1, fp32-exact),
          plus a 2x-mode fp16 tensor_tensor max into run2 (dist2 path)
ACT absorbs the PSUM->SBUF conversion so the DVE only spends
one full-rate PSUM scan plus one half-rate fp16 pass per half-tile.
Repetition is idempotent (max-accumulators see identical data each rep),
so only the loop bound changes with `reps` and the timed marginal cost is
pure device execution.

Device outputs raw row maxes [128, 64] and unfolded fp16 column maxes
[128, 4096]; the host (O(N) work) folds partitions, does relu/sqrt, means,
and exact top-k via np.partition, then averages the 8 losses.
"""

import os
import sys

import numpy as np

for _p in ("/opt/trn_rl_repo",):
    if os.path.isdir(_p) and _p not in sys.path:
        sys.path.insert(0, _p)

import concourse.bass as bass  # noqa: E402
import concourse.mybir as mybir  # noqa: E402
import concourse.tile as tile  # noqa: E402
from concourse import bacc  # noqa: E402
from concourse.bass import ds, ts  # noqa: E402
from concourse.bass_utils import run_bass_kernel_spmd  # noqa: E402

B = 8
N = 4096  # pred points per batch
M = 4096  # gt points per batch
P = 128  # partitions
KSLOTS = 13
NTILE = N // P  # 32
MCHUNK = 2048  # gt columns per PSUM half
NCHUNK = M // MCHUNK  # 2
MM_N = 512  # moving free dim per matmul (<= 1 PSUM bank)
K1 = N // 2  # top-k count (PERCENT=0.5)
WEIGHT = 3.0

F16 = mybir.dt.float16
F32 = mybir.dt.float32
Alu = mybir.AluOpType
X = mybir.AxisListType.X

LAST_RESULT = None
_CACHE = {}


def _build_nc(reps=1):
    nc = bacc.Bacc(
        "TRN2", target_bir_lowering=False, debug=False, num_devices=B
    )
    a_in = nc.dram_tensor("A", [KSLOTS, N], F16, kind="ExternalInput")
    g_in = nc.dram_tensor("G", [KSLOTS, M], F16, kind="ExternalInput")
    r1_out = nc.dram_tensor("R1OUT", [P, NTILE * NCHUNK], F32, kind="ExternalOutput")
    c2_out = nc.dram_tensor("C2OUT", [P, M], F16, kind="ExternalOutput")

    with tile.TileContext(nc) as tc:
        _body(nc, tc, a_in, g_in, r1_out, c2_out, reps)
    nc.compile()
    return nc


def _body(nc, tc, a_in, g_in, r1_out, c2_out, reps):
    from contextlib import ExitStack

    with ExitStack() as ctx:
        io = ctx.enter_context(tc.tile_pool(name="io", bufs=1))
        cpool = ctx.enter_context(tc.tile_pool(name="c16", bufs=2))

        A = io.tile([KSLOTS, N], F16)
        G = io.tile([KSLOTS, M], F16)
        nc.sync.dma_start(out=A, in_=a_in[:])
        nc.sync.dma_start(out=G, in_=g_in[:])

        run2 = io.tile([P, M], F16)
        nc.vector.memset(run2, -60000.0)
        Rpart = io.tile([P, NTILE * NCHUNK], F32, name="Rpart")

        with tc.tile_pool(name="ps_main", bufs=2, space="PSUM") as psum:
            with tc.For_i(0, reps, 1) as _r:
                for i in range(NTILE):
                    for j in range(NCHUNK):
                        ps = psum.tile([P, MCHUNK], F32)
                        for kk in range(MCHUNK // MM_N):
                            c0 = j * MCHUNK + kk * MM_N
                            nc.tensor.matmul(
                                ps[:, kk * MM_N : (kk + 1) * MM_N],
                                A[:, i * P : (i + 1) * P],
                                G[:, c0 : c0 + MM_N],
                                start=True,
                                stop=True,
                            )
                        s = i * NCHUNK + j
                        # ACT converts the half to fp16 while DVE reduces it
                        C = cpool.tile([P, MCHUNK], F16)
                        nc.scalar.copy(C, ps)
                        nc.vector.tensor_reduce(
                            out=Rpart[:, s : s + 1],
                            in_=ps,
                            axis=X,
                            op=Alu.max,
                        )
                        # colmax accumulate (dist2) in fp16 2x mode
                        nc.vector.tensor_tensor(
                            run2[:, j * MCHUNK : (j + 1) * MCHUNK],
                            C,
                            run2[:, j * MCHUNK : (j + 1) * MCHUNK],
                            op=Alu.max,
                        )

        nc.sync.dma_start(out=r1_out[:], in_=Rpart)
        nc.sync.dma_start(out=c2_out[:], in_=run2)


def _split16(x):
    hi = x.astype(np.float16)
    lo = (x - hi.astype(np.float64)).astype(np.float16)
    return hi, lo


def _prep(pred, gt):
    """Build the [13, 4096] fp16 stationary/moving operand matrices."""
    p = pred.astype(np.float64)
    g = gt.astype(np.float64)
    ph, pl = _split16(p)  # [N,3] each
    gh, gl = _split16(g)
    pt = ph.astype(np.float64) + pl.astype(np.float64)
    gt_ = gh.astype(np.float64) + gl.astype(np.float64)
    pn = (pt * pt).sum(-1)  # [N]
    gn = (gt_ * gt_).sum(-1)  # [M]
    pnh, pnl = _split16(-pn)
    gnh, gnl = _split16(-gn)

    A = np.zeros((KSLOTS, N), np.float16)
    G = np.zeros((KSLOTS, M), np.float16)
    for c in range(3):
        r = 3 * c
        # (ph+pl)*(gh+gl) ~= ph*gh + ph*gl + pl*gh  (pl*gl ~ 2^-22, dropped)
        A[r + 0] = 2.0 * ph[:, c]
        A[r + 1] = 2.0 * ph[:, c]
        A[r + 2] = 2.0 * pl[:, c]
        G[r + 0] = gh[:, c]
        G[r + 1] = gl[:, c]
        G[r + 2] = gh[:, c]
    A[9] = pnh
    A[10] = pnl
    G[9] = 1.0
    G[10] = 1.0
    A[11] = 1.0
    A[12] = 1.0
    G[11] = gnh
    G[12] = gnl
    return A, G


def _get_nc():
    if "nc" not in _CACHE:
        _CACHE["nc"] = _build_nc()
    return _CACHE["nc"]


def kernel(pred_pc, gt_pc):
    global LAST_RESULT
    pred_pc = np.asarray(pred_pc)
    gt_pc = np.asarray(gt_pc)
    nc = _get_nc()
    in_maps = []
    for b in range(B):
        A, G = _prep(pred_pc[b], gt_pc[b])
        in_maps.append({"A": A, "G": G})
    res = run_bass_kernel_spmd(nc, in_maps, list(range(B)))
    LAST_RESULT = res
    losses = []
    for b in range(B):
        r1 = np.asarray(res.results[b]["R1OUT"], np.float32)
        c2 = np.asarray(res.results[b]["C2OUT"], np.float32).max(axis=0)  # [4096]
        # rowmax: combine NCHUNK partials per point
        r1 = r1.reshape(P, NTILE, NCHUNK).max(axis=2)  # [P, NTILE]
        d1 = np.sqrt(np.maximum(-r1, 0.0)).reshape(-1)  # [4096]
        d2 = np.sqrt(np.maximum(-c2, 0.0))  # [4096]
        loss = 0.0
        for d in (d1, d2):
            topk = np.partition(d, d.size - K1)[d.size - K1 :]
            loss += d.mean() + WEIGHT * topk.mean()
        losses.append(loss)
    return np.array(np.mean(losses), dtype=np.float32)


# revision 17
# speedup vs baseline: 103.6943x; 15.6852x over previous
"""Trainium2 Bass kernel: batched chamfer-style metric (nn_Metric_56985626083917).

Reference computation per batch b (B=8, N=M=4096, D=3):
    sqd[n,m] = |pred_n - gt_m|^2   (clamped >= 0)
    dist1 = sqrt(min_m sqd)  [N] ; dist2 = sqrt(min_n sqd)  [M]
    loss_b = mean(dist1)+mean(dist2) + 3*(mean(top2048(dist1))+mean(top2048(dist2)))
    out = mean_b loss_b

Strategy: data-parallel, one batch per NeuronCore (8 cores).
Per core the device computes zt[n,m] = -sqd[n,m] via K=13 fp16 matmuls with
error-compensated hi/lo splits (fp32-grade accuracy at full fp16 PE rate):
    zt = sum_c 2*p_c*g_c - |p|^2 - |g|^2
slots: per coord c 3 products (ph*gh, ph*gl, pl*gh; pl*gl ~ 2^-22 dropped),
plus 2 slots for -|p|^2 (hi/lo vs ones) and 2 slots for -|g|^2 -> K=13.

Main loop: statically unrolled over 32 pred tiles x 2 PSUM halves of 2048
gt columns (double-buffered). No hardware loop: tc.For_i executes at
~5-8us per instruction on this stack (measured; likely SW-decode), so
full unrolling is ~7x faster despite the larger NEFF. Benchmark reps are
additional unrolled copies of the body (V0 scheme):
    PE  : 4 matmuls -> PSUM [128, 2048] fp32 (zt half-tile)
    ACT : copy PSUM -> SBUF fp16 (feeds the dist2 path)
    DVE : tensor_reduce max over PSUM -> rowmax column (dist1, fp32-exact),
          plus a 2x-mode fp16 tensor_tensor max into run2 (dist2 path)
ACT absorbs the PSUM->SBUF conversion so the DVE only spends
one full-rate PSUM scan plus one half-rate fp16 pass per half-tile.
Repetition is idempotent (max-accumulators see identical data each rep),
so only the loop bound changes with `reps` and the timed marginal cost is
pure device execution.

Device outputs raw row maxes [128, 64] and unfolded fp16 column maxes
[128, 4096]; the host (O(N) work) folds partitions, does relu/sqrt, means,
and exact top-k via np.partition, then averages the 8 losses.
"""

import os
import sys

import numpy as np

for _p in ("/opt/trn_rl_repo",):
    if os.path.isdir(_p) and _p not in sys.path:
        sys.path.insert(0, _p)

import concourse.bass as bass  # noqa: E402
import concourse.bass_isa as bass_isa  # noqa: E402
import concourse.mybir as mybir  # noqa: E402
import concourse.tile as tile  # noqa: E402
from concourse import bacc  # noqa: E402
from concourse.bass_utils import run_bass_kernel_spmd  # noqa: E402

B = 8
N = 4096  # pred points per batch
M = 4096  # gt points per batch
P = 128  # partitions
KSLOTS = 13
NTILE = N // P  # 32
MCHUNK = 2048  # gt columns per PSUM half
NCHUNK = M // MCHUNK  # 2
MM_N = 512  # moving free dim per matmul (<= 1 PSUM bank)
K1 = N // 2  # top-k count (PERCENT=0.5)
WEIGHT = 3.0

F16 = mybir.dt.float16
F32 = mybir.dt.float32
Alu = mybir.AluOpType
X = mybir.AxisListType.X

LAST_RESULT = None
_CACHE = {}


def _build_nc(reps=1):
    nc = bacc.Bacc(
        "TRN2", target_bir_lowering=False, debug=False, num_devices=B
    )
    a_in = nc.dram_tensor("A", [KSLOTS, N], F16, kind="ExternalInput")
    g_in = nc.dram_tensor("G", [KSLOTS, M], F16, kind="ExternalInput")
    r1_out = nc.dram_tensor("R1OUT", [P, NTILE * NCHUNK], F32, kind="ExternalOutput")
    c2_out = nc.dram_tensor("C2OUT", [1, M], F16, kind="ExternalOutput")

    with tile.TileContext(nc) as tc:
        for _ in range(reps):
            _body(nc, tc, a_in, g_in, r1_out, c2_out)
    nc.compile()
    return nc


def _body(nc, tc, a_in, g_in, r1_out, c2_out):
    from contextlib import ExitStack

    with ExitStack() as ctx:
        io = ctx.enter_context(tc.tile_pool(name="io", bufs=1))
        cpool = ctx.enter_context(tc.tile_pool(name="c16", bufs=2))

        A = io.tile([KSLOTS, N], F16)
        G = io.tile([KSLOTS, M], F16)
        nc.sync.dma_start(out=A, in_=a_in[:])
        nc.sync.dma_start(out=G, in_=g_in[:])

        run2 = io.tile([P, M], F16)
        nc.vector.memset(run2, -60000.0)
        Rpart = io.tile([P, NTILE * NCHUNK], F32, name="Rpart")

        with tc.tile_pool(name="ps_main", bufs=2, space="PSUM") as psum:
            for i in range(NTILE):
                for j in range(NCHUNK):
                    ps = psum.tile([P, MCHUNK], F32)
                    for kk in range(MCHUNK // MM_N):
                        c0 = j * MCHUNK + kk * MM_N
                        nc.tensor.matmul(
                            ps[:, kk * MM_N : (kk + 1) * MM_N],
                            A[:, i * P : (i + 1) * P],
                            G[:, c0 : c0 + MM_N],
                            start=True,
                            stop=True,
                        )
                    s = i * NCHUNK + j
                    # ACT converts the half to fp16 while DVE reduces it
                    C = cpool.tile([P, MCHUNK], F16)
                    nc.scalar.copy(C, ps)
                    nc.vector.tensor_reduce(
                        out=Rpart[:, s : s + 1],
                        in_=ps,
                        axis=X,
                        op=Alu.max,
                    )
                    # colmax accumulate (dist2) in fp16 2x mode
                    nc.vector.tensor_tensor(
                        run2[:, j * MCHUNK : (j + 1) * MCHUNK],
                        C,
                        run2[:, j * MCHUNK : (j + 1) * MCHUNK],
                        op=Alu.max,
                    )

        # fold run2 across partitions on-device (gpsimd all-reduce) so only
        # [1, M] goes back over the wire
        foldt = io.tile([P, M], F16, name="foldt")
        nc.gpsimd.partition_all_reduce(foldt, run2, P, bass_isa.ReduceOp.max)

        nc.sync.dma_start(out=r1_out[:], in_=Rpart)
        nc.sync.dma_start(out=c2_out[:], in_=foldt[0:1, :])


def _split16(x):
    hi = x.astype(np.float16)
    lo = (x - hi.astype(np.float64)).astype(np.float16)
    return hi, lo


def _prep(pred, gt):
    """Build the [13, 4096] fp16 stationary/moving operand matrices."""
    p = pred.astype(np.float64)
    g = gt.astype(np.float64)
    ph, pl = _split16(p)  # [N,3] each
    gh, gl = _split16(g)
    pt = ph.astype(np.float64) + pl.astype(np.float64)
    gt_ = gh.astype(np.float64) + gl.astype(np.float64)
    pn = (pt * pt).sum(-1)  # [N]
    gn = (gt_ * gt_).sum(-1)  # [M]
    pnh, pnl = _split16(-pn)
    gnh, gnl = _split16(-gn)

    A = np.zeros((KSLOTS, N), np.float16)
    G = np.zeros((KSLOTS, M), np.float16)
    for c in range(3):
        r = 3 * c
        # (ph+pl)*(gh+gl) ~= ph*gh + ph*gl + pl*gh  (pl*gl ~ 2^-22, dropped)
        A[r + 0] = 2.0 * ph[:, c]
        A[r + 1] = 2.0 * ph[:, c]
        A[r + 2] = 2.0 * pl[:, c]
        G[r + 0] = gh[:, c]
        G[r + 1] = gl[:, c]
        G[r + 2] = gh[:, c]
    A[9] = pnh
    A[10] = pnl
    G[9] = 1.0
    G[10] = 1.0
    A[11] = 1.0
    A[12] = 1.0
    G[11] = gnh
    G[12] = gnl
    return A, G


def _get_nc():
    if "nc" not in _CACHE:
        _CACHE["nc"] = _build_nc()
    return _CACHE["nc"]


def kernel(pred_pc, gt_pc):
    global LAST_RESULT
    pred_pc = np.asarray(pred_pc)
    gt_pc = np.asarray(gt_pc)
    nc = _get_nc()
    in_maps = []
    for b in range(B):
        A, G = _prep(pred_pc[b], gt_pc[b])
        in_maps.append({"A": A, "G": G})
    res = run_bass_kernel_spmd(nc, in_maps, list(range(B)))
    LAST_RESULT = res
    losses = []
    for b in range(B):
        r1 = np.asarray(res.results[b]["R1OUT"], np.float32)
        c2 = np.asarray(res.results[b]["C2OUT"], np.float32).reshape(M)  # [4096]
        # rowmax: combine NCHUNK partials per point
        r1 = r1.reshape(P, NTILE, NCHUNK).max(axis=2)  # [P, NTILE]
        d1 = np.sqrt(np.maximum(-r1, 0.0)).reshape(-1)  # [4096]
        d2 = np.sqrt(np.maximum(-c2, 0.0))  # [4096]
        loss = 0.0
        for d in (d1, d2):
            topk = np.partition(d, d.size - K1)[d.size - K1 :]
            loss += d.mean() + WEIGHT * topk.mean()
        losses.append(loss)
    return np.array(np.mean(losses), dtype=np.float32)


# revision 18
# speedup vs baseline: 111.8234x; 1.0784x over previous
"""Trainium2 Bass kernel: batched chamfer-style metric (nn_Metric_56985626083917).

Reference computation per batch b (B=8, N=M=4096, D=3):
    sqd[n,m] = |pred_n - gt_m|^2   (clamped >= 0)
    dist1 = sqrt(min_m sqd)  [N] ; dist2 = sqrt(min_n sqd)  [M]
    loss_b = mean(dist1)+mean(dist2) + 3*(mean(top2048(dist1))+mean(top2048(dist2)))
    out = mean_b loss_b

Strategy: data-parallel, one batch per NeuronCore (8 cores).
Per core the device computes zt[n,m] = -sqd[n,m] via K=13 fp16 matmuls with
error-compensated hi/lo splits (fp32-grade accuracy at full fp16 PE rate):
    zt = sum_c 2*p_c*g_c - |p|^2 - |g|^2
slots: per coord c 3 products (ph*gh, ph*gl, pl*gh; pl*gl ~ 2^-22 dropped),
plus 2 slots for -|p|^2 (hi/lo vs ones) and 2 slots for -|g|^2 -> K=13.

Main loop: statically unrolled over 32 pred tiles x 2 PSUM halves of 2048
gt columns (double-buffered). No hardware loop: tc.For_i executes at
~5-8us per instruction on this stack (measured; likely SW-decode), so
full unrolling is ~7x faster despite the larger NEFF. Benchmark reps are
additional unrolled copies of the body (V0 scheme):
    PE  : 4 matmuls -> PSUM [128, 2048] fp32 (zt half-tile)
    ACT : copy PSUM -> SBUF fp16 (feeds the dist2 path)
    DVE : tensor_reduce max over PSUM -> rowmax column (dist1, fp32-exact),
          plus a 2x-mode fp16 tensor_tensor max into run2 (dist2 path)
ACT absorbs the PSUM->SBUF conversion so the DVE only spends
one full-rate PSUM scan plus one half-rate fp16 pass per half-tile.

Device outputs raw row maxes [128, 64] fp32 and the column maxes folded
across partitions on-device (gpsimd partition_all_reduce) to [1, 4096]
fp16 — keeping transfers tiny; the host (O(N) work) does relu/sqrt,
means, and exact top-k via np.partition, then averages the 8 losses.
"""

import os
import sys

import numpy as np

for _p in ("/opt/trn_rl_repo",):
    if os.path.isdir(_p) and _p not in sys.path:
        sys.path.insert(0, _p)

import concourse.bass as bass  # noqa: E402
import concourse.bass_isa as bass_isa  # noqa: E402
import concourse.mybir as mybir  # noqa: E402
import concourse.tile as tile  # noqa: E402
from concourse import bacc  # noqa: E402
from concourse.bass_utils import run_bass_kernel_spmd  # noqa: E402

B = 8
N = 4096  # pred points per batch
M = 4096  # gt points per batch
P = 128  # partitions
KSLOTS = 13
NTILE = N // P  # 32
MCHUNK = 2048  # gt columns per PSUM half
NCHUNK = M // MCHUNK  # 2
MM_N = 512  # moving free dim per matmul (<= 1 PSUM bank)
K1 = N // 2  # top-k count (PERCENT=0.5)
WEIGHT = 3.0

F16 = mybir.dt.float16
F32 = mybir.dt.float32
Alu = mybir.AluOpType
X = mybir.AxisListType.X

LAST_RESULT = None
_CACHE = {}


def _build_nc(reps=1):
    nc = bacc.Bacc(
        "TRN2", target_bir_lowering=False, debug=False, num_devices=B
    )
    a_in = nc.dram_tensor("A", [KSLOTS, N], F16, kind="ExternalInput")
    g_in = nc.dram_tensor("G", [KSLOTS, M], F16, kind="ExternalInput")
    r1_out = nc.dram_tensor("R1OUT", [P, NTILE * NCHUNK], F32, kind="ExternalOutput")
    c2_out = nc.dram_tensor("C2OUT", [1, M], F16, kind="ExternalOutput")

    with tile.TileContext(nc) as tc:
        for _ in range(reps):
            _body(nc, tc, a_in, g_in, r1_out, c2_out)
    nc.compile()
    return nc


def _body(nc, tc, a_in, g_in, r1_out, c2_out):
    from contextlib import ExitStack

    with ExitStack() as ctx:
        io = ctx.enter_context(tc.tile_pool(name="io", bufs=1))
        cpool = ctx.enter_context(tc.tile_pool(name="c16", bufs=2))

        A = io.tile([KSLOTS, N], F16)
        G = io.tile([KSLOTS, M], F16)
        nc.sync.dma_start(out=A, in_=a_in[:])
        nc.sync.dma_start(out=G, in_=g_in[:])

        run2 = io.tile([P, M], F16)
        nc.vector.memset(run2, -60000.0)
        Rpart = io.tile([P, NTILE * NCHUNK], F32, name="Rpart")

        with tc.tile_pool(name="ps_main", bufs=2, space="PSUM") as psum:
            for i in range(NTILE):
                for j in range(NCHUNK):
                    ps = psum.tile([P, MCHUNK], F32)
                    for kk in range(MCHUNK // MM_N):
                        c0 = j * MCHUNK + kk * MM_N
                        nc.tensor.matmul(
                            ps[:, kk * MM_N : (kk + 1) * MM_N],
                            A[:, i * P : (i + 1) * P],
                            G[:, c0 : c0 + MM_N],
                            start=True,
                            stop=True,
                        )
                    s = i * NCHUNK + j
                    # ACT converts the half to fp16 while DVE reduces it
                    C = cpool.tile([P, MCHUNK], F16)
                    nc.scalar.copy(C, ps)
                    nc.vector.tensor_reduce(
                        out=Rpart[:, s : s + 1],
                        in_=ps,
                        axis=X,
                        op=Alu.max,
                    )
                    # colmax accumulate (dist2) in fp16 2x mode
                    nc.vector.tensor_tensor(
                        run2[:, j * MCHUNK : (j + 1) * MCHUNK],
                        C,
                        run2[:, j * MCHUNK : (j + 1) * MCHUNK],
                        op=Alu.max,
                    )

        # fold run2 across partitions on-device (gpsimd all-reduce) so only
        # [1, M] goes back over the wire
        foldt = io.tile([P, M], F16, name="foldt")
        nc.gpsimd.partition_all_reduce(foldt, run2, P, bass_isa.ReduceOp.max)

        nc.sync.dma_start(out=r1_out[:], in_=Rpart)
        nc.sync.dma_start(out=c2_out[:], in_=foldt[0:1, :])


def _split16(x):
    hi = x.astype(np.float16)
    lo = (x - hi.astype(np.float64)).astype(np.float16)
    return hi, lo


def _prep(pred, gt):
    """Build the [13, 4096] fp16 stationary/moving operand matrices."""
    p = pred.astype(np.float64)
    g = gt.astype(np.float64)
    ph, pl = _split16(p)  # [N,3] each
    gh, gl = _split16(g)
    pt = ph.astype(np.float64) + pl.astype(np.float64)
    gt_ = gh.astype(np.float64) + gl.astype(np.float64)
    pn = (pt * pt).sum(-1)  # [N]
    gn = (gt_ * gt_).sum(-1)  # [M]
    pnh, pnl = _split16(-pn)
    gnh, gnl = _split16(-gn)

    A = np.zeros((KSLOTS, N), np.float16)
    G = np.zeros((KSLOTS, M), np.float16)
    for c in range(3):
        r = 3 * c
        # (ph+pl)*(gh+gl) ~= ph*gh + ph*gl + pl*gh  (pl*gl ~ 2^-22, dropped)
        A[r + 0] = 2.0 * ph[:, c]
        A[r + 1] = 2.0 * ph[:, c]
        A[r + 2] = 2.0 * pl[:, c]
        G[r + 0] = gh[:, c]
        G[r + 1] = gl[:, c]
        G[r + 2] = gh[:, c]
    A[9] = pnh
    A[10] = pnl
    G[9] = 1.0
    G[10] = 1.0
    A[11] = 1.0
    A[12] = 1.0
    G[11] = gnh
    G[12] = gnl
    return A, G


def _get_nc():
    if "nc" not in _CACHE:
        _CACHE["nc"] = _build_nc()
    return _CACHE["nc"]


def kernel(pred_pc, gt_pc):
    global LAST_RESULT
    pred_pc = np.asarray(pred_pc)
    gt_pc = np.asarray(gt_pc)
    nc = _get_nc()
    in_maps = []
    for b in range(B):
        A, G = _prep(pred_pc[b], gt_pc[b])
        in_maps.append({"A": A, "G": G})
    res = run_bass_kernel_spmd(nc, in_maps, list(range(B)))
    LAST_RESULT = res
    losses = []
    for b in range(B):
        r1 = np.asarray(res.results[b]["R1OUT"], np.float32)
        c2 = np.asarray(res.results[b]["C2OUT"], np.float32).reshape(M)  # [4096]
        # rowmax: combine NCHUNK partials per point
        r1 = r1.reshape(P, NTILE, NCHUNK).max(axis=2)  # [P, NTILE]
        d1 = np.sqrt(np.maximum(-r1, 0.0)).reshape(-1)  # [4096]
        d2 = np.sqrt(np.maximum(-c2, 0.0))  # [4096]
        loss = 0.0
        for d in (d1, d2):
            topk = np.partition(d, d.size - K1)[d.size - K1 :]
            loss += d.mean() + WEIGHT * topk.mean()
        losses.append(loss)
    return np.array(np.mean(losses), dtype=np.float32)
